# revision 1
# baseline (speedup 1.0000x reference)
"""CorrelationSampler Trainium2 kernel.

out[b, h, w, c] = bilinear sample of corr[b, :, :, c] at grid position
(h + flow_y, w + flow_x)-ish (align_corners=True, border padding).

Strategy:
  - Host computes integer corner indices and the 4 bilinear weights per
    output position (cheap: B*H*W = 16K positions).
  - Corner indices are re-clamped so ix1 == ix0+1 always (ix0 <= W-2),
    which is mathematically identical to the reference clipping and makes
    the two x-neighbors one contiguous 2*4096-float chunk in memory.
  - 8 cores = batch (4) x position-half (2). Each core gathers row-pairs
    of its batch's [4096, 4096] correlation matrix with indirect DMA and
    blends them on the vector engine with per-partition scalar weights.
"""

import numpy as np

B, H, W = 4, 64, 64
HW = H * W  # 4096 channels; also 4096 source rows per batch
N_CORES = 8
POS_PER_CORE = (B * HW) // N_CORES  # 2048
P = 128  # partitions
N_TILES = POS_PER_CORE // P  # 16


def _host_indices_weights(flow: np.ndarray):
    """float32 replica of the reference's grid math -> corner row indices
    and bilinear corner weights, shape [B, H*W] each."""
    f32 = np.float32
    y_g, x_g = np.meshgrid(
        np.arange(H, dtype=f32), np.arange(W, dtype=f32), indexing="ij"
    )
    x_norm = (f32(2.0) * x_g / f32(W - 1) - f32(1.0)).astype(f32)
    y_norm = (f32(2.0) * y_g / f32(H - 1) - f32(1.0)).astype(f32)

    fx = flow[:, 0].astype(f32)
    fy = flow[:, 1].astype(f32)
    gx = x_norm[None] + fx / f32(W) * f32(2.0)
    gy = y_norm[None] + fy / f32(H) * f32(2.0)

    ix = np.clip((gx + f32(1.0)) * f32(0.5) * f32(W - 1), f32(0.0), f32(W - 1))
    iy = np.clip((gy + f32(1.0)) * f32(0.5) * f32(H - 1), f32(0.0), f32(H - 1))

    # floor is >= 0 after the clip; clamp to W-2/H-2 so the +1 neighbor
    # always exists. At the high border this gives weight 1.0 on the last
    # row/col -- identical result to the reference's clip formulation.
    ix0 = np.minimum(np.floor(ix), f32(W - 2)).astype(np.int32)
    iy0 = np.minimum(np.floor(iy), f32(H - 2)).astype(np.int32)
    wx = (ix - ix0.astype(f32)).astype(f32)
    wy = (iy - iy0.astype(f32)).astype(f32)

    one = f32(1.0)
    w00 = ((one - wy) * (one - wx)).astype(f32)
    w01 = ((one - wy) * wx).astype(f32)
    w10 = (wy * (one - wx)).astype(f32)
    w11 = (wy * wx).astype(f32)

    row0 = iy0 * np.int32(W) + ix0  # gather start row for (iy0, ix0..ix0+1)
    row1 = row0 + np.int32(W)  # (iy0+1, ix0..ix0+1)

    flat = lambda a: a.reshape(B, HW)
    return (
        flat(row0),
        flat(row1),
        flat(w00),
        flat(w01),
        flat(w10),
        flat(w11),
    )


def _build_program():
    import concourse.bacc as bacc
    import concourse.bass as bass
    import concourse.mybir as mybir
    from concourse.tile import TileContext

    f32 = mybir.dt.float32
    i32 = mybir.dt.int32

    nc = bacc.Bacc(
        "TRN2", target_bir_lowering=False, debug=False, num_devices=N_CORES
    )
    corr = nc.dram_tensor("corr", [HW, HW], f32, kind="ExternalInput").ap()
    idx = nc.dram_tensor("idx", [P, 2 * N_TILES + 1], i32, kind="ExternalInput").ap()
    wts = nc.dram_tensor("wts", [P, 4 * N_TILES], f32, kind="ExternalInput").ap()
    out = nc.dram_tensor(
        "out", [POS_PER_CORE, HW], f32, kind="ExternalOutput"
    ).ap()

    mult = mybir.AluOpType.mult
    add = mybir.AluOpType.add

    with TileContext(nc) as tc:
        with (
            tc.tile_pool(name="meta", bufs=1) as meta,
            tc.tile_pool(name="pairs", bufs=2) as pairp,
            tc.tile_pool(name="acc", bufs=4) as accp,
        ):
            idx_t = meta.tile([P, 2 * N_TILES + 1], i32)
            wts_t = meta.tile([P, 4 * N_TILES], f32)
            # idx via gpsimd: same engine as the gathers, avoids a
            # cross-engine semaphore hop on the critical startup path
            nc.gpsimd.dma_start(out=idx_t[:], in_=idx[:])
            nc.sync.dma_start(out=wts_t[:], in_=wts[:])

            for t in range(N_TILES):
                # Two indirect gathers per tile (one per y-row): each
                # partition reads 8192 contiguous floats = source rows
                # (y, x0) and (y, x0+1) -> pair[p] = [a | b] slabs.
                pair0 = pairp.tile([P, 2 * HW], f32, tag="pair0")
                pair1 = pairp.tile([P, 2 * HW], f32, tag="pair1")
                nc.gpsimd.indirect_dma_start(
                    out=pair0[:],
                    out_offset=None,
                    in_=corr[:],
                    in_offset=bass.IndirectOffsetOnAxis(
                        ap=idx_t[:, 2 * t : 2 * t + 1], axis=0
                    ),
                )
                if t < N_TILES - 1:
                    nc.gpsimd.indirect_dma_start(
                        out=pair1[:],
                        out_offset=None,
                        in_=corr[:],
                        in_offset=bass.IndirectOffsetOnAxis(
                            ap=idx_t[:, 2 * t + 1 : 2 * t + 2], axis=0
                        ),
                    )
                else:
                    # split the kernel's final gather: row1 then row1+1,
                    # so the last HBM dependency is half-size and the
                    # closing blend+store starts sooner
                    nc.gpsimd.indirect_dma_start(
                        out=pair1[:, 0:HW],
                        out_offset=None,
                        in_=corr[:],
                        in_offset=bass.IndirectOffsetOnAxis(
                            ap=idx_t[:, 2 * t + 1 : 2 * t + 2], axis=0
                        ),
                    )
                    nc.gpsimd.indirect_dma_start(
                        out=pair1[:, HW : 2 * HW],
                        out_offset=None,
                        in_=corr[:],
                        in_offset=bass.IndirectOffsetOnAxis(
                            ap=idx_t[:, 2 * N_TILES : 2 * N_TILES + 1], axis=0
                        ),
                    )
                w = [wts_t[:, k * N_TILES + t : k * N_TILES + t + 1] for k in range(4)]
                slabs = [pair0, pair0, pair1, pair1]
                # Last tile: blend+store in two channel chunks so the final
                # store overlaps the final blend (shorter pipeline drain).
                n_chunks = 2 if t == N_TILES - 1 else 1
                csz = HW // n_chunks
                for c0 in range(0, HW, csz):
                    acc = accp.tile([P, csz], f32, tag="acc")
                    sl = lambda k: slabs[k][:, (k % 2) * HW + c0 : (k % 2) * HW + c0 + csz]
                    # acc = w00*a + w01*b + w10*c + w11*d
                    nc.vector.tensor_scalar_mul(acc[:], sl(0), w[0])
                    for k in range(1, 4):
                        nc.vector.scalar_tensor_tensor(
                            acc[:], sl(k), w[k], acc[:], mult, add
                        )
                    nc.sync.dma_start(
                        out=out[t * P : (t + 1) * P, c0 : c0 + csz], in_=acc[:]
                    )
    nc.compile()
    return nc


def _core_meta(row0, row1, w00, w01, w10, w11, b, half):
    """Pack per-core idx [P, 2*N_TILES] and wts [P, 4*N_TILES] tensors.

    Core (b, half) handles flat positions [half*2048, (half+1)*2048) of
    batch b. Positions are sorted by gather address (row0) before being
    assigned to (tile, partition) slots: consecutive descriptors then hit
    adjacent/duplicate source rows, which raises the DRAM row-buffer hit
    rate of the random gather stream. The device writes results in sorted
    order; `perm` lets the host scatter rows back at unshard time."""
    sl = slice(half * POS_PER_CORE, (half + 1) * POS_PER_CORE)
    perm = np.argsort(row0[b, sl], kind="stable")
    # [POS_PER_CORE] sorted -> [N_TILES, P] -> [P, N_TILES]
    tp = lambda a: np.ascontiguousarray(a[b, sl][perm].reshape(N_TILES, P).T)
    # idx columns interleaved (row0_t, row1_t) so tile t's offset AP is
    # the [P, 2] slice idx[:, 2t:2t+2]
    idx = np.empty((P, 2 * N_TILES + 1), dtype=np.int32)
    r1 = tp(row1)
    idx[:, 0:-1:2] = tp(row0)
    idx[:, 1:-1:2] = r1
    idx[:, -1] = r1[:, -1] + 1  # last tile's row1+1 for the split gather
    wts = np.concatenate(
        [tp(w00), tp(w01), tp(w10), tp(w11)], axis=1
    ).astype(np.float32)
    return np.ascontiguousarray(idx), np.ascontiguousarray(wts), perm


_cached = {}


def _get_program():
    if "nc" not in _cached:
        _cached["nc"] = _build_program()
    return _cached["nc"]


def _ensure_axon_hooks_importable():
    """bass_utils imports antenv.axon_hooks when tracing is requested (e.g.
    BASS_TRACE=1). Some containers ship an antenv stub without that module;
    provide a no-op registry so tracing degrades gracefully instead of
    crashing the run."""
    import sys
    import types

    try:
        import antenv.axon_hooks  # noqa: F401
    except Exception:
        m = types.ModuleType("antenv.axon_hooks")
        m._hook = None
        m.set_axon_ntff_profile_hook = lambda h: setattr(m, "_hook", h)
        m.get_axon_ntff_profile_hook = lambda: getattr(m, "_hook", None)
        sys.modules["antenv.axon_hooks"] = m


def kernel(correlation: np.ndarray, flow: np.ndarray, _trace: bool = False):
    _ensure_axon_hooks_importable()
    from concourse.bass_utils import run_bass_kernel_spmd

    correlation = np.ascontiguousarray(correlation, dtype=np.float32)
    flow = np.asarray(flow, dtype=np.float32)

    row0, row1, w00, w01, w10, w11 = _host_indices_weights(flow)

    in_maps = []
    perms = []
    for core in range(N_CORES):
        b, half = divmod(core, 2)
        idx, wts, perm = _core_meta(row0, row1, w00, w01, w10, w11, b, half)
        perms.append(perm)
        in_maps.append(
            {
                "corr": correlation[b].reshape(HW, HW),
                "idx": idx,
                "wts": wts,
            }
        )

    nc = _get_program()
    extra = {"trace_cores": list(range(N_CORES))} if _trace else {}
    res = run_bass_kernel_spmd(
        nc, in_maps, core_ids=list(range(N_CORES)), trace=_trace, **extra
    )

    out = np.empty((B, HW, HW), dtype=np.float32)
    for core in range(N_CORES):
        b, half = divmod(core, 2)
        # device rows are in address-sorted order; scatter back to
        # natural position order
        out[b, half * POS_PER_CORE + perms[core], :] = res.results[core]["out"]
    if _trace:
        kernel.last_results = res
    return out.reshape(B, H, W, HW)



# revision 3
# speedup vs baseline: 3.2591x; 3.2591x over previous
"""CorrelationSampler Trainium2 kernel — band-matmul formulation.

out[b, p, c] = sum of 4 bilinear corner weights * corr[b, corner_row(p), c]
            = (S_b @ corr_b)[p, c]

where S_b is a [4096, 4096] sparse matrix with 4 nonzeros per row at
columns {r, r+1, r+64, r+65} (r = iy0*64+ix0 per output position).

Key idea: the naive gather reads every correlation row ~4x (4 corners per
position, rows shared between positions). Casting to bf16 and sorting the
output positions by base row r makes S band-diagonal: each 128-position
tile only touches ~2-3 adjacent 128-row tiles of corr. The TensorEngine
then computes out = S @ corr with corr streamed from HBM exactly ONCE.

Per-core HBM traffic drops from ~160 MB (fp32 gather baseline) to ~35 MB:
  16 MB corr band (bf16) + 3 MB S blocks + 16 MB output (bf16).

Sharding: 8 cores = 4 batches x 2 channel-halves (each core: all 4096
positions of one batch, 2048 of the 4096 channels). S depends only on
flow, so the two halves of a batch share the same S. bf16 is safe: the
tolerance is 2e-2 and bf16 end-to-end error is ~0.5e-2 worst-case.
"""

import numpy as np
import ml_dtypes

BF16 = np.dtype(ml_dtypes.bfloat16)

B, H, W = 4, 64, 64
HW = H * W  # 4096
N_CORES = 8
P = 128
N_PTILES = HW // P  # 32 position tiles (all positions, sorted)
CH_PER_CORE = HW // 2  # 2048 channels per core
N_CHUNK = 512  # matmul free dim (one PSUM bank of fp32)
N_CHUNKS = CH_PER_CORE // N_CHUNK  # 4
STORE_GROUP = 8  # pos-tiles buffered per output store (1 MB stores)


def _host_indices_weights(flow: np.ndarray):
    """float32 replica of the reference's grid math -> base corner row
    index row0 and the 4 bilinear corner weights, shape [B, H*W] each.
    Corner rows of position p are row0, row0+1, row0+64, row0+65."""
    f32 = np.float32
    y_g, x_g = np.meshgrid(
        np.arange(H, dtype=f32), np.arange(W, dtype=f32), indexing="ij"
    )
    x_norm = (f32(2.0) * x_g / f32(W - 1) - f32(1.0)).astype(f32)
    y_norm = (f32(2.0) * y_g / f32(H - 1) - f32(1.0)).astype(f32)

    fx = flow[:, 0].astype(f32)
    fy = flow[:, 1].astype(f32)
    gx = x_norm[None] + fx / f32(W) * f32(2.0)
    gy = y_norm[None] + fy / f32(H) * f32(2.0)

    ix = np.clip((gx + f32(1.0)) * f32(0.5) * f32(W - 1), f32(0.0), f32(W - 1))
    iy = np.clip((gy + f32(1.0)) * f32(0.5) * f32(H - 1), f32(0.0), f32(H - 1))

    # floor >= 0 after the clip; clamp to W-2/H-2 so the +1 neighbor always
    # exists (at the high border all weight lands on the last row/col --
    # identical to the reference's clip formulation).
    ix0 = np.minimum(np.floor(ix), f32(W - 2)).astype(np.int32)
    iy0 = np.minimum(np.floor(iy), f32(H - 2)).astype(np.int32)
    wx = (ix - ix0.astype(f32)).astype(f32)
    wy = (iy - iy0.astype(f32)).astype(f32)

    one = f32(1.0)
    w00 = ((one - wy) * (one - wx)).astype(f32)
    w01 = ((one - wy) * wx).astype(f32)
    w10 = (wy * (one - wx)).astype(f32)
    w11 = (wy * wx).astype(f32)

    row0 = iy0 * np.int32(W) + ix0
    flat = lambda a: a.reshape(B, HW)
    return flat(row0), flat(w00), flat(w01), flat(w10), flat(w11)


def _windows(radius):
    """Static per-pos-tile source-tile windows (flow independent so all 8
    SPMD cores share one program)."""
    wins = []
    for k in range(N_PTILES):
        wins.append(list(range(max(0, k - radius), min(N_PTILES - 1, k + radius) + 1)))
    return wins


def _build_program(radius):
    import concourse.bacc as bacc
    import concourse.mybir as mybir
    from concourse.tile import TileContext

    bf16 = mybir.dt.bfloat16
    f32 = mybir.dt.float32

    wins = _windows(radius)
    nmm = sum(len(w) for w in wins)

    nc = bacc.Bacc(
        "TRN2", target_bir_lowering=False, debug=False, num_devices=N_CORES
    )
    # band[p, s, ch]: corr row s*128+p, channel ch (this core's half)
    band = nc.dram_tensor(
        "band", [P, N_PTILES, CH_PER_CORE], bf16, kind="ExternalInput"
    ).ap()
    # s_all[:, j*128:(j+1)*128] = j-th stationary block S^T[src_row, pos]
    s_all = nc.dram_tensor(
        "s_all", [P, nmm * P], bf16, kind="ExternalInput"
    ).ap()
    # out[p, k, ch]: sorted position k*128+p
    out = nc.dram_tensor(
        "out", [P, N_PTILES, CH_PER_CORE], bf16, kind="ExternalOutput"
    ).ap()

    with TileContext(nc) as tc:
        with (
            tc.tile_pool(name="meta", bufs=1) as meta,
            tc.tile_pool(name="bandp", bufs=2) as bandp,
            tc.tile_pool(name="outp", bufs=2) as outp,
            tc.tile_pool(name="psum", bufs=4, space="PSUM") as psump,
        ):
            s_t = meta.tile([P, nmm * P], bf16)
            nc.sync.dma_start(out=s_t[:], in_=s_all[:])

            for c in range(N_CHUNKS):
                bt = bandp.tile([P, N_PTILES, N_CHUNK], bf16, tag="band")
                nc.sync.dma_start(
                    out=bt[:], in_=band[:, :, c * N_CHUNK : (c + 1) * N_CHUNK]
                )
                flat = 0
                for k0 in range(0, N_PTILES, STORE_GROUP):
                    ot = outp.tile([P, STORE_GROUP, N_CHUNK], bf16, tag="out")
                    for j in range(STORE_GROUP):
                        k = k0 + j
                        win = wins[k]
                        ps = psump.tile([P, N_CHUNK], f32, tag="ps")
                        for i, s in enumerate(win):
                            nc.tensor.matmul(
                                ps[:],
                                s_t[:, (flat + i) * P : (flat + i + 1) * P],
                                bt[:, s, :],
                                start=(i == 0),
                                stop=(i == len(win) - 1),
                            )
                        flat += len(win)
                        nc.vector.tensor_copy(out=ot[:, j, :], in_=ps[:])
                    nc.sync.dma_start(
                        out=out[
                            :, k0 : k0 + STORE_GROUP, c * N_CHUNK : (c + 1) * N_CHUNK
                        ],
                        in_=ot[:],
                    )
    nc.compile()
    return nc


def _plan(flow):
    """Sort positions by base row per batch, pick the minimal window
    radius covering every corner, and build the S^T stationary blocks."""
    row0, w00, w01, w10, w11 = _host_indices_weights(flow)

    perms = []
    row0s_list = []
    radius = 1
    for b in range(B):
        perm = np.argsort(row0[b], kind="stable")
        perms.append(perm)
        r0s = row0[b][perm]
        row0s_list.append(r0s)
        tiles = r0s.reshape(N_PTILES, P)
        lo = tiles.min(axis=1)  # min corner row per pos-tile
        hi = tiles.max(axis=1) + W + 1  # max corner row (row0+65)
        k = np.arange(N_PTILES)
        need = max(int(np.max(k - lo // P)), int(np.max(hi // P - k)))
        radius = max(radius, need)

    wins = _windows(radius)
    nmm = sum(len(w) for w in wins)

    s_blocks = []  # per batch: [P, nmm*P] bf16
    for b in range(B):
        r0s = row0s_list[b]
        ws = [a[b][perms[b]] for a in (w00, w01, w10, w11)]
        corners = [r0s, r0s + 1, r0s + W, r0s + W + 1]
        s_all = np.zeros((P, nmm * P), dtype=np.float32)
        flat = 0
        pos_local = np.tile(np.arange(P), N_PTILES).reshape(N_PTILES, P)
        for k in range(N_PTILES):
            sl = slice(k * P, (k + 1) * P)
            for i, s in enumerate(wins[k]):
                blk = s_all[:, (flat + i) * P : (flat + i + 1) * P]
                base = s * P
                for g_all, w_all in zip(corners, ws):
                    g = g_all[sl] - base
                    w = w_all[sl]
                    m = (g >= 0) & (g < P)
                    np.add.at(blk, (g[m], pos_local[k][m]), w[m])
            flat += len(wins[k])
        s_blocks.append(np.ascontiguousarray(s_all.astype(BF16)))

    return radius, perms, s_blocks


_cached = {}


def _get_program(radius):
    key = ("nc", radius)
    if key not in _cached:
        _cached[key] = _build_program(radius)
    return _cached[key]


def _ensure_axon_hooks_importable():
    """bass_utils imports antenv.axon_hooks when tracing is requested.
    Some containers ship an antenv stub without that module; provide a
    no-op registry so tracing degrades gracefully instead of crashing."""
    import sys
    import types

    try:
        import antenv.axon_hooks  # noqa: F401
    except Exception:
        m = types.ModuleType("antenv.axon_hooks")
        m._hook = None
        m.set_axon_ntff_profile_hook = lambda h: setattr(m, "_hook", h)
        m.get_axon_ntff_profile_hook = lambda: getattr(m, "_hook", None)
        sys.modules["antenv.axon_hooks"] = m


def kernel(correlation: np.ndarray, flow: np.ndarray, _trace: bool = False):
    _ensure_axon_hooks_importable()
    from concourse.bass_utils import run_bass_kernel_spmd

    correlation = np.asarray(correlation, dtype=np.float32)
    flow = np.asarray(flow, dtype=np.float32)

    radius, perms, s_blocks = _plan(flow)

    in_maps = []
    for core in range(N_CORES):
        b, half = divmod(core, 2)
        ch0 = half * CH_PER_CORE
        # band[p, s, ch] = corr[b, s*128+p, ch0+ch]
        band = (
            correlation[b]
            .reshape(HW, HW)[:, ch0 : ch0 + CH_PER_CORE]
            .astype(BF16)
            .reshape(N_PTILES, P, CH_PER_CORE)
            .transpose(1, 0, 2)
        )
        in_maps.append(
            {
                "band": np.ascontiguousarray(band),
                "s_all": s_blocks[b],
            }
        )

    nc = _get_program(radius)
    extra = {"trace_cores": list(range(N_CORES))} if _trace else {}
    res = run_bass_kernel_spmd(
        nc, in_maps, core_ids=list(range(N_CORES)), trace=_trace, **extra
    )

    out = np.empty((B, HW, HW), dtype=np.float32)
    for b in range(B):
        halves = [
            np.asarray(res.results[2 * b + h]["out"]).astype(np.float32)
            for h in range(2)
        ]
        # [P, N_PTILES, 4096] -> sorted-position-major [4096, 4096]
        full = np.concatenate(halves, axis=2).transpose(1, 0, 2).reshape(HW, HW)
        out[b, perms[b], :] = full
    if _trace:
        kernel.last_results = res
    return out.reshape(B, H, W, HW)


# revision 5
# speedup vs baseline: 3.7272x; 1.1436x over previous
"""CorrelationSampler Trainium2 kernel — band-matmul formulation.

out[b, p, c] = sum of 4 bilinear corner weights * corr[b, corner_row(p), c]
            = (S_b @ corr_b)[p, c]

where S_b is a [4096, 4096] sparse matrix with 4 nonzeros per row at
columns {r, r+1, r+64, r+65} (r = iy0*64+ix0 per output position).

Key idea: the naive gather reads every correlation row ~4x (4 corners per
position, rows shared between positions). Casting to bf16 and sorting the
output positions by base row r makes S band-diagonal: each 128-position
tile only touches ~2-3 adjacent 128-row tiles of corr. The TensorEngine
then computes out = S @ corr with corr streamed from HBM exactly ONCE.

Per-core HBM traffic drops from ~160 MB (fp32 gather baseline) to ~35 MB:
  16 MB corr band (bf16) + 3 MB S blocks + 16 MB output (bf16).

Sharding: 8 cores = 4 batches x 2 channel-halves (each core: all 4096
positions of one batch, 2048 of the 4096 channels). S depends only on
flow, so the two halves of a batch share the same S. bf16 is safe: the
tolerance is 2e-2 and bf16 end-to-end error is ~0.5e-2 worst-case.
"""

import numpy as np
import ml_dtypes

BF16 = np.dtype(ml_dtypes.bfloat16)

B, H, W = 4, 64, 64
HW = H * W  # 4096
N_CORES = 8
P = 128
N_PTILES = HW // P  # 32 position tiles (all positions, sorted)
CH_PER_CORE = HW // 2  # 2048 channels per core
N_CHUNK = 512  # matmul free dim (one PSUM bank of fp32)
N_CHUNKS = CH_PER_CORE // N_CHUNK  # 4
STORE_GROUP = 4  # pos-tiles buffered per output store (512 KB stores)
SUB = 8  # source tiles per band sub-load (1 MB sub-loads)
N_SUBS = N_PTILES // SUB  # 4 band sub-tiles per chunk


def _host_indices_weights(flow: np.ndarray):
    """float32 replica of the reference's grid math -> base corner row
    index row0 and the 4 bilinear corner weights, shape [B, H*W] each.
    Corner rows of position p are row0, row0+1, row0+64, row0+65."""
    f32 = np.float32
    y_g, x_g = np.meshgrid(
        np.arange(H, dtype=f32), np.arange(W, dtype=f32), indexing="ij"
    )
    x_norm = (f32(2.0) * x_g / f32(W - 1) - f32(1.0)).astype(f32)
    y_norm = (f32(2.0) * y_g / f32(H - 1) - f32(1.0)).astype(f32)

    fx = flow[:, 0].astype(f32)
    fy = flow[:, 1].astype(f32)
    gx = x_norm[None] + fx / f32(W) * f32(2.0)
    gy = y_norm[None] + fy / f32(H) * f32(2.0)

    ix = np.clip((gx + f32(1.0)) * f32(0.5) * f32(W - 1), f32(0.0), f32(W - 1))
    iy = np.clip((gy + f32(1.0)) * f32(0.5) * f32(H - 1), f32(0.0), f32(H - 1))

    # floor >= 0 after the clip; clamp to W-2/H-2 so the +1 neighbor always
    # exists (at the high border all weight lands on the last row/col --
    # identical to the reference's clip formulation).
    ix0 = np.minimum(np.floor(ix), f32(W - 2)).astype(np.int32)
    iy0 = np.minimum(np.floor(iy), f32(H - 2)).astype(np.int32)
    wx = (ix - ix0.astype(f32)).astype(f32)
    wy = (iy - iy0.astype(f32)).astype(f32)

    one = f32(1.0)
    w00 = ((one - wy) * (one - wx)).astype(f32)
    w01 = ((one - wy) * wx).astype(f32)
    w10 = (wy * (one - wx)).astype(f32)
    w11 = (wy * wx).astype(f32)

    row0 = iy0 * np.int32(W) + ix0
    flat = lambda a: a.reshape(B, HW)
    return flat(row0), flat(w00), flat(w01), flat(w10), flat(w11)


def _windows(radius):
    """Static per-pos-tile source-tile windows (flow independent so all 8
    SPMD cores share one program)."""
    wins = []
    for k in range(N_PTILES):
        wins.append(list(range(max(0, k - radius), min(N_PTILES - 1, k + radius) + 1)))
    return wins


def _build_program(radius):
    import concourse.bacc as bacc
    import concourse.mybir as mybir
    from concourse.tile import TileContext

    bf16 = mybir.dt.bfloat16
    f32 = mybir.dt.float32

    wins = _windows(radius)
    nmm = sum(len(w) for w in wins)

    nc = bacc.Bacc(
        "TRN2", target_bir_lowering=False, debug=False, num_devices=N_CORES
    )
    # band[p, s, ch]: corr row s*128+p, channel ch (this core's half)
    band = nc.dram_tensor(
        "band", [P, N_PTILES, CH_PER_CORE], bf16, kind="ExternalInput"
    ).ap()
    # s_all[:, j*128:(j+1)*128] = j-th stationary block S^T[src_row, pos]
    s_all = nc.dram_tensor(
        "s_all", [P, nmm * P], bf16, kind="ExternalInput"
    ).ap()
    # out[p, k, ch]: sorted position k*128+p
    out = nc.dram_tensor(
        "out", [P, N_PTILES, CH_PER_CORE], bf16, kind="ExternalOutput"
    ).ap()

    # S piece boundaries (split the stationary upload so early matmuls
    # don't wait on the whole 3 MB block tensor)
    n_pieces = 4
    bounds = [round(i * nmm / n_pieces) for i in range(n_pieces + 1)]

    def s_piece(j):  # flat block index -> (piece, local col range)
        for pi in range(n_pieces):
            if bounds[pi] <= j < bounds[pi + 1]:
                return pi, j - bounds[pi]
        raise AssertionError(j)

    with TileContext(nc) as tc:
        with (
            tc.tile_pool(name="meta", bufs=1) as meta,
            tc.tile_pool(name="bandp", bufs=3 * N_SUBS) as bandp,
            tc.tile_pool(name="outp", bufs=3) as outp,
            tc.tile_pool(name="psum", bufs=6, space="PSUM") as psump,
        ):
            s_ts = []
            for pi in range(n_pieces):
                npc = bounds[pi + 1] - bounds[pi]
                st = meta.tile([P, npc * P], bf16, tag=f"s{pi}")
                nc.sync.dma_start(
                    out=st[:], in_=s_all[:, bounds[pi] * P : bounds[pi + 1] * P]
                )
                s_ts.append(st)

            for c in range(N_CHUNKS):
                bts = []
                for g in range(N_SUBS):
                    bt = bandp.tile([P, SUB, N_CHUNK], bf16, tag="band")
                    nc.sync.dma_start(
                        out=bt[:],
                        in_=band[
                            :, g * SUB : (g + 1) * SUB, c * N_CHUNK : (c + 1) * N_CHUNK
                        ],
                    )
                    bts.append(bt)
                flat = 0
                for k0 in range(0, N_PTILES, STORE_GROUP):
                    ot = outp.tile([P, STORE_GROUP, N_CHUNK], bf16, tag="out")
                    for j in range(STORE_GROUP):
                        k = k0 + j
                        win = wins[k]
                        ps = psump.tile([P, N_CHUNK], f32, tag="ps")
                        for i, s in enumerate(win):
                            pi, loc = s_piece(flat + i)
                            nc.tensor.matmul(
                                ps[:],
                                s_ts[pi][:, loc * P : (loc + 1) * P],
                                bts[s // SUB][:, s % SUB, :],
                                start=(i == 0),
                                stop=(i == len(win) - 1),
                            )
                        flat += len(win)
                        nc.vector.tensor_copy(out=ot[:, j, :], in_=ps[:])
                    nc.sync.dma_start(
                        out=out[
                            :, k0 : k0 + STORE_GROUP, c * N_CHUNK : (c + 1) * N_CHUNK
                        ],
                        in_=ot[:],
                    )
    nc.compile()
    return nc


def _plan(flow):
    """Sort positions by base row per batch, pick the minimal window
    radius covering every corner, and build the S^T stationary blocks."""
    row0, w00, w01, w10, w11 = _host_indices_weights(flow)

    perms = []
    row0s_list = []
    radius = 1
    for b in range(B):
        perm = np.argsort(row0[b], kind="stable")
        perms.append(perm)
        r0s = row0[b][perm]
        row0s_list.append(r0s)
        tiles = r0s.reshape(N_PTILES, P)
        lo = tiles.min(axis=1)  # min corner row per pos-tile
        hi = tiles.max(axis=1) + W + 1  # max corner row (row0+65)
        k = np.arange(N_PTILES)
        need = max(int(np.max(k - lo // P)), int(np.max(hi // P - k)))
        radius = max(radius, need)

    wins = _windows(radius)
    nmm = sum(len(w) for w in wins)

    s_blocks = []  # per batch: [P, nmm*P] bf16
    for b in range(B):
        r0s = row0s_list[b]
        ws = [a[b][perms[b]] for a in (w00, w01, w10, w11)]
        corners = [r0s, r0s + 1, r0s + W, r0s + W + 1]
        s_all = np.zeros((P, nmm * P), dtype=np.float32)
        flat = 0
        pos_local = np.tile(np.arange(P), N_PTILES).reshape(N_PTILES, P)
        for k in range(N_PTILES):
            sl = slice(k * P, (k + 1) * P)
            for i, s in enumerate(wins[k]):
                blk = s_all[:, (flat + i) * P : (flat + i + 1) * P]
                base = s * P
                for g_all, w_all in zip(corners, ws):
                    g = g_all[sl] - base
                    w = w_all[sl]
                    m = (g >= 0) & (g < P)
                    np.add.at(blk, (g[m], pos_local[k][m]), w[m])
            flat += len(wins[k])
        s_blocks.append(np.ascontiguousarray(s_all.astype(BF16)))

    return radius, perms, s_blocks


_cached = {}


def _get_program(radius):
    key = ("nc", radius)
    if key not in _cached:
        _cached[key] = _build_program(radius)
    return _cached[key]


def _ensure_axon_hooks_importable():
    """bass_utils imports antenv.axon_hooks when tracing is requested.
    Some containers ship an antenv stub without that module; provide a
    no-op registry so tracing degrades gracefully instead of crashing."""
    import sys
    import types

    try:
        import antenv.axon_hooks  # noqa: F401
    except Exception:
        m = types.ModuleType("antenv.axon_hooks")
        m._hook = None
        m.set_axon_ntff_profile_hook = lambda h: setattr(m, "_hook", h)
        m.get_axon_ntff_profile_hook = lambda: getattr(m, "_hook", None)
        sys.modules["antenv.axon_hooks"] = m


def kernel(correlation: np.ndarray, flow: np.ndarray, _trace: bool = False):
    _ensure_axon_hooks_importable()
    from concourse.bass_utils import run_bass_kernel_spmd

    correlation = np.asarray(correlation, dtype=np.float32)
    flow = np.asarray(flow, dtype=np.float32)

    radius, perms, s_blocks = _plan(flow)

    in_maps = []
    for core in range(N_CORES):
        b, half = divmod(core, 2)
        ch0 = half * CH_PER_CORE
        # band[p, s, ch] = corr[b, s*128+p, ch0+ch]
        band = (
            correlation[b]
            .reshape(HW, HW)[:, ch0 : ch0 + CH_PER_CORE]
            .astype(BF16)
            .reshape(N_PTILES, P, CH_PER_CORE)
            .transpose(1, 0, 2)
        )
        in_maps.append(
            {
                "band": np.ascontiguousarray(band),
                "s_all": s_blocks[b],
            }
        )

    nc = _get_program(radius)
    extra = {"trace_cores": list(range(N_CORES))} if _trace else {}
    res = run_bass_kernel_spmd(
        nc, in_maps, core_ids=list(range(N_CORES)), trace=_trace, **extra
    )

    out = np.empty((B, HW, HW), dtype=np.float32)
    for b in range(B):
        halves = [
            np.asarray(res.results[2 * b + h]["out"]).astype(np.float32)
            for h in range(2)
        ]
        # [P, N_PTILES, 4096] -> sorted-position-major [4096, 4096]
        full = np.concatenate(halves, axis=2).transpose(1, 0, 2).reshape(HW, HW)
        out[b, perms[b], :] = full
    if _trace:
        kernel.last_results = res
    return out.reshape(B, H, W, HW)
